# revision 23
# baseline (speedup 1.0000x reference)
"""Trainium2 Bass kernel for nn_AttentionBlock (B=4, C=128, T=4096, K=64, V=128).

Sharding: 8 cores = 4 batches x 2 j-groups (data parallel over batch, plus a
split of the key/value axis j; the host sums the two partial read matrices).

Design notes (v2, restructured for ScalarE-bound overlap):
- The kernel is fundamentally bound by exp() on the Scalar engine
  (1 col/cycle @ 1.2 GHz, ~34.8K cols/core ~= 29 us). Everything else
  (PE ~20 us, DVE ~15 us, DMA ~5 us) is organized to hide under it.
- Host pre-casts x / weights to bf16: halves input DMA and removes all
  on-device cast traffic.
- Q^T / K^T are built with row-duplicated weights ([Wq|Wq]) so the two
  512-wide i-chunk QK^T matmuls (contraction K=64) can run CONCURRENTLY
  in the PE array via row tiling (rows 0-63 vs 64-127).
- Diagonal i-chunk of each j-tile is trimmed: the ACTIVATE (exp) spans only
  256 cols (even tiles) instead of 512; the dead tail of e is pre-zeroed.
- PSUM: tag "qk" 2 x [128,1536] (ping-pong: PE fills one group while
  ScalarE exps the other) + tag "ro" 2 x [128,512] (projections, V, readout).
- Output is DMA'd in bf16; host accumulates in f32.
"""

import numpy as np

_B, _C, _T = 4, 128, 4096
_K, _V = 64, 128
_JT = 16          # local 128-wide j tiles per core -> 2048 local j columns
_CH = 512         # i-chunk width (one PSUM bank in fp32)

_NEG = -1.0e30    # effective -inf for the causal mask (exp -> 0 exactly)
_LACT = (256, 512)  # activation span in the diagonal chunk, by tile parity

_cache = {}


def _build_nc():
    from contextlib import ExitStack

    import concourse.tile as tile
    from concourse import bacc, mybir
    from concourse.masks import make_identity

    f32 = mybir.dt.float32
    bf16 = mybir.dt.bfloat16
    AF = mybir.ActivationFunctionType

    nc = bacc.Bacc("TRN2", target_bir_lowering=False)

    # packed inputs: one DMA apiece.
    # wpk (bf16): [wq2 | wk2 | wv | mask0 | mask1] = 128+128+128+512+512 cols
    # fpk (f32):  [bq2 | bk2 | bvb] = 1+1+512 cols
    xb_d = nc.dram_tensor("xb", [_C, _T], bf16, kind="ExternalInput")
    xj_d = nc.dram_tensor("xj", [_C, _JT * 128], bf16, kind="ExternalInput")
    wpk_d = nc.dram_tensor("wpk", [128, 1408], bf16, kind="ExternalInput")
    fpk_d = nc.dram_tensor("fpk", [128, 514], f32, kind="ExternalInput")
    out_d = nc.dram_tensor("out", [_V, _T], bf16, kind="ExternalOutput")

    with tile.TileContext(nc) as tc, ExitStack() as ctx:
        singles = ctx.enter_context(tc.tile_pool(name="singles", bufs=1))
        work = ctx.enter_context(tc.tile_pool(name="work", bufs=2))
        small = ctx.enter_context(tc.tile_pool(name="small", bufs=4))
        psum = ctx.enter_context(tc.tile_pool(name="psum", bufs=1, space="PSUM"))

        # trigger the ACT table load immediately (it otherwise fires right
        # before the first real activation, serializing the pre-loop)
        warm0 = singles.tile([128, 1], f32)
        nc.vector.memset(warm0, 0.0)
        warm1 = singles.tile([128, 1], f32)
        nc.scalar.activation(warm1, warm0, AF.Exp)

        # ---------------- input DMAs (already bf16) ----------------
        # spread across engine queues so the transfers run on parallel rings
        wpk = singles.tile([128, 1408], bf16)
        nc.sync.dma_start(out=wpk, in_=wpk_d[:])
        fpk = singles.tile([128, 514], f32)
        nc.sync.dma_start(out=fpk, in_=fpk_d[:])
        xb_bf = singles.tile([_C, _T], bf16)
        xj_bf = singles.tile([_C, _JT * 128], bf16)
        for c in range(2):
            nc.sync.dma_start(out=xb_bf[:, c * 1024:(c + 1) * 1024],
                              in_=xb_d[:, c * 1024:(c + 1) * 1024])
        for c in (1, 0):
            nc.gpsimd.dma_start(out=xj_bf[:, c * 1024:(c + 1) * 1024],
                                in_=xj_d[:, c * 1024:(c + 1) * 1024])
        for c in (2, 3):
            nc.gpsimd.dma_start(out=xb_bf[:, c * 1024:(c + 1) * 1024],
                                in_=xb_d[:, c * 1024:(c + 1) * 1024])

        wq_bf = wpk[:, 0:128]
        wk_bf = wpk[:, 128:256]
        wv_bf = wpk[:, 256:384]
        mask0 = wpk[:, 384:896]
        mask1 = wpk[:, 896:1408]
        bq_s = fpk[:, 0:1]
        bk_s = fpk[:, 1:2]
        bvb_s = fpk[:, 2:514]

        id_bf = singles.tile([128, 128], bf16)
        make_identity(nc, id_bf[:])

        # ---------------- projections ----------------
        # qt[0:64] = Q^T, qt[64:128] = Q^T again (row-duplicated weights) so
        # QK^T matmuls can be row-tiled pairwise.
        qt_bf = singles.tile([128, _T], bf16)
        for gi, (g0, w) in enumerate(((0, 3), (3, 3), (6, 2))):
            ps = psum.tile([128, 1536], f32, tag="qk", bufs=2, name="ps_qt")
            for j in range(w):
                c = g0 + j
                nc.tensor.matmul(ps[:, j * _CH:(j + 1) * _CH], wq_bf,
                                 xb_bf[:, c * _CH:(c + 1) * _CH],
                                 start=True, stop=True)
            if gi == 0:
                nc.scalar.add(
                    qt_bf[:, g0 * _CH:(g0 + w) * _CH], ps[:, 0:w * _CH], bq_s)
            else:
                nc.vector.tensor_scalar_add(
                    qt_bf[:, g0 * _CH:(g0 + w) * _CH], ps[:, 0:w * _CH],
                    bq_s)

        kt_bf = singles.tile([128, _JT * 128], bf16)
        # high chunk first: k=15 needs kt cols [1920:2048] as soon as possible
        ps = psum.tile([128, _CH], f32, tag="ro", bufs=2, name="ps_kt_hi")
        nc.tensor.matmul(ps[:], wk_bf, xj_bf[:, 3 * _CH:4 * _CH],
                         start=True, stop=True)
        nc.scalar.add(kt_bf[:, 3 * _CH:4 * _CH], ps[:], bk_s)
        ps = psum.tile([128, 1536], f32, tag="qk", bufs=2, name="ps_kt_lo")
        for c in range(3):
            nc.tensor.matmul(ps[:, c * _CH:(c + 1) * _CH], wk_bf,
                             xj_bf[:, c * _CH:(c + 1) * _CH],
                             start=True, stop=True)
        nc.vector.tensor_scalar_add(kt_bf[:, 0:3 * _CH], ps[:, 0:3 * _CH],
                                    bk_s)

        # v[jl, v] = x_j^T Wv + bv, per 128-wide j-tile; 4 tiles per PSUM buf.
        # The bv broadcast tile ([128, 4*V] f32, bv tiled 4x) comes from the
        # host so the bias is a free part of the PSUM->SBUF move (TT-add).
        v_f32 = singles.tile([128, _JT, _V], f32)
        for i in range(4):
            ps = psum.tile([128, _CH], f32, tag="ro", bufs=2, name="ps_v")
            for j in range(4):
                kk = 4 * i + j
                nc.tensor.matmul(ps[:, j * _V:(j + 1) * _V],
                                 xj_bf[:, kk * 128:(kk + 1) * 128],
                                 wv_bf, start=True, stop=True)
            nc.vector.tensor_add(v_f32[:, 4 * i:4 * i + 4, :], ps[:],
                                 bvb_s)

        # ---------------- attention ----------------
        e_all = singles.tile([128, _JT, _T], bf16)
        vs_bf = singles.tile([128, _JT, _V], bf16)

        # pre-zero the dead tail of each even tile's diagonal chunk
        # (the exp ACTIVATE only covers the first _LACT[0] cols there)
        for k in range(0, _JT, 2):
            d = k // 2 + 1
            nc.gpsimd.memset(
                e_all[:, k, (d - 1) * _CH + _LACT[0]:d * _CH], 0.0)

        # Readout work is drip-fed: each chunk's (16-2c) accumulation matmuls
        # are emitted a few at a time between QK groups, sized to the PE
        # slack under that group's exp, so ScalarE never starves behind a
        # monolithic readout block and the PE has no long idle gaps.
        # Up to two chunks accumulate concurrently (the two "ro" PSUM slots);
        # a task (c, kk) is eligible during iteration k only if kk > k, so an
        # emitted matmul never stalls the PE on a not-yet-computed vs tile.
        ro_pending = list(range(7, -1, -1))   # chunks, descending
        ro_open = []                          # [{c, ps, idx, tasks}]

        def ro_refill():
            while len(ro_open) < 2 and ro_pending:
                c = ro_pending.pop(0)
                ro_open.append({
                    "c": c,
                    "ps": psum.tile([128, _CH], f32, tag="ro", bufs=2,
                                    name="ps_ro"),
                    "idx": 0,
                    "tasks": list(range(_JT - 1, 2 * c - 1, -1)),
                })

        def ro_emit(n, k):
            """Emit up to n eligible readout matmuls (tasks with kk > k)."""
            ro_refill()
            while n > 0:
                st = next((s for s in ro_open if s["tasks"][s["idx"]] > k),
                          None)
                if st is None:
                    return
                c, i = st["c"], st["idx"]
                kk = st["tasks"][i]
                last = (i == len(st["tasks"]) - 1)
                nc.tensor.matmul(st["ps"][0:_V, :], vs_bf[:, kk, :],
                                 e_all[:, kk, c * _CH:(c + 1) * _CH],
                                 start=(i == 0), stop=last)
                st["idx"] += 1
                n -= 1
                if last:
                    ot = work.tile([_V, _CH], bf16, tag="osb")
                    nc.vector.tensor_copy(ot, st["ps"][0:_V, :])
                    nc.sync.dma_start(out=out_d[:, c * _CH:(c + 1) * _CH],
                                      in_=ot)
                    ro_open.remove(st)
                    ro_refill()
        for k in range(_JT - 1, -1, -1):
            d = k // 2 + 1
            r = k % 2
            Ld = _LACT[r]
            accs = []
            if k == _JT - 1:
                bounds = [(0, 2), (2, 5), (5, 8)]
            else:
                bounds = [(gs, min(gs + 3, d)) for gs in range(0, d, 3)]
            for gs, ge in bounds:
                ps = psum.tile([128, 1536], f32, tag="qk", bufs=2, name="ps_qk")
                diag_in_group = (ge == d)
                for c in range(gs, ge):
                    off = (c - gs) * _CH
                    diag = (c == d - 1)
                    N = Ld if diag else _CH
                    h = 64 * (c % 2)
                    nc.tensor.matmul(
                        ps[:, off:off + N],
                        kt_bf[h:h + 64, k * 128:(k + 1) * 128],
                        qt_bf[h:h + 64, c * _CH:c * _CH + N],
                        start=True, stop=not diag)
                    if diag:
                        mk = mask1 if r else mask0
                        nc.tensor.matmul(ps[:, off:off + N], id_bf,
                                         mk[:, 0:N],
                                         start=False, stop=True)
                fd = (ge - 1 - gs) * _CH + (Ld if ge == d else _CH)
                acc = small.tile([128, 1], f32, tag="acc", bufs=6)
                nc.scalar.activation(out=e_all[:, k, gs * _CH:gs * _CH + fd],
                                     in_=ps[:, 0:fd],
                                     func=AF.Exp, scale=0.125, accum_out=acc)
                accs.append(acc)
                # fill the PE slack under this group's exp with readout work
                scal_ns = fd / 1.2 + 550
                qk_ns = 350 * ((ge - gs + 1) // 2) + (260 if diag_in_group
                                                      else 0)
                n_ro = int(max(0, min(4, round((scal_ns - qk_ns) / 360))))
                ro_emit(n_ro, k)
            s_t = accs[0]
            for a in accs[1:]:
                s_new = small.tile([128, 1], f32, tag="s", bufs=2)
                nc.vector.tensor_add(s_new, s_t, a)
                s_t = s_new
            rs = small.tile([128, 1], f32, tag="rs", bufs=2)
            nc.vector.reciprocal(rs, s_t)
            nc.vector.tensor_scalar_mul(vs_bf[:, k, :], v_f32[:, k, :], rs)

        while ro_open:
            ro_emit(100, -1)

    nc.compile()
    return nc


def _get_nc():
    if "nc" not in _cache:
        _cache["nc"] = _build_nc()
    return _cache["nc"]


def _masks(g):
    """Additive causal-mask tiles (bf16) for a core in j-group g.

    Tile r (= local j-tile parity) masks the diagonal 512-wide i-chunk of
    every local j-tile with that parity: entry [p, ii] is live iff
    global_i <= global_j, i.e. ii <= (j0 - i0) + p with j0 - i0 = 128g + 256r.
    """
    import ml_dtypes

    m = np.zeros((2, 128, _CH), np.float32)
    p = np.arange(128)[:, None]
    ii = np.arange(_CH)[None, :]
    for parity in range(2):
        o = 128 * g + 256 * parity
        m[parity] = np.where(ii <= o + p, 0.0, _NEG)
    return m.astype(ml_dtypes.bfloat16)


def kernel(**inputs):
    import ml_dtypes

    from concourse.bass_utils import run_bass_kernel_spmd

    bf16 = ml_dtypes.bfloat16

    x = np.asarray(inputs["x"], dtype=np.float32)
    Wq = np.asarray(inputs["Wq"], dtype=np.float32)
    Wk = np.asarray(inputs["Wk"], dtype=np.float32)
    Wv = np.asarray(inputs["Wv"], dtype=np.float32)
    bq = np.asarray(inputs["bq"], dtype=np.float32).reshape(_K)
    bk = np.asarray(inputs["bk"], dtype=np.float32).reshape(_K)
    bv = np.asarray(inputs["bv"], dtype=np.float32).reshape(1, _V)

    xbf = x.astype(bf16)
    wq2 = np.concatenate([Wq, Wq], axis=1)          # [128, 128]
    wk2 = np.concatenate([Wk, Wk], axis=1)
    bq2 = np.concatenate([bq, bq]).reshape(128, 1)
    bk2 = np.concatenate([bk, bk]).reshape(128, 1)
    bvb = np.broadcast_to(np.tile(bv, (1, 4)), (128, 4 * _V))
    fpk = np.ascontiguousarray(np.concatenate(
        [np.broadcast_to(bq2, (128, 1)), np.broadcast_to(bk2, (128, 1)), bvb],
        axis=1, dtype=np.float32))                  # [128, 514]

    nc = _get_nc()
    in_maps = []
    for core in range(8):
        b, g = divmod(core, 2)
        # this core's j columns: tiles {2k+g}, i.e. starts 256k + 128g
        cols = ((np.arange(_JT) * 256 + 128 * g)[:, None]
                + np.arange(128)[None, :]).ravel()
        mk = _masks(g)
        wpk = np.ascontiguousarray(np.concatenate(
            [wq2.astype(bf16), wk2.astype(bf16), Wv.astype(bf16),
             mk[0], mk[1]], axis=1))                # [128, 1408] bf16
        in_maps.append({
            "xb": np.ascontiguousarray(xbf[b]),
            "xj": np.ascontiguousarray(xbf[b][:, cols]),
            "wpk": wpk, "fpk": fpk,
        })

    trace = bool(_cache.get("trace"))
    res = run_bass_kernel_spmd(nc, in_maps, core_ids=list(range(8)),
                               trace=trace)
    _cache["last_result"] = res

    parts = [r["out"] for r in res.results]
    out = np.empty((_B, _C + _V, _T), np.float32)
    for b in range(_B):
        out[b, :_C] = x[b]
        out[b, _C:] = (parts[2 * b].astype(np.float32)
                       + parts[2 * b + 1].astype(np.float32))
    return out
